# revision 37
# baseline (speedup 1.0000x reference)
"""Causal self-attention (B=2, T=4096, C=512, H=8, Dh=64) on 8 trn2 cores.

Sharding: core = (batch, head-pair). 2 batches x 4 head-pairs = 8 cores.
Each core computes q/k/v projections for its 2 heads, causal attention in
S^T ([k, q]) layout, and a row-parallel slice of the output projection.
Host sums the 4 partial outputs per batch (+ b_out) and stacks batches.

bf16 pipeline (PSUM accumulation stays f32 where it matters):
  - x / weights arrive bf16; Q/K/V produced bf16 (projection matmuls
    accumulate f32 in PSUM, DVE bias-add casts to bf16).
  - S^T = KT-chunk.T @ QT written to a bf16 PSUM tile [128, 2, 2, 512]
    covering BOTH heads of a chunk pair -> ONE exp activation per pair
    at [128, 2048] (amortizes ACT's +352cyc/instr overhead).
  - Causal mask: only the [128,128] triangle block of each diagonal
    chunk is multiplied (DVE bf16); the fully-masked 128r-column prefix
    is excluded by narrowing the Y matmul instead of zeroing.
  - YT[h][65, 512] += V_chunk @ expS in f32 PSUM (row 64 = softmax
    denominator via an appended ones column in V).
  - Deferred normalization: yt_ps evicted immediately (bf16 YTu + f32r
    den row) so the next tile's Y matmuls never wait on the reciprocal;
    recip -> PE partition-broadcast -> DVE multiply runs one tile behind,
    interleaved with the next tile's attention, as does the row-parallel
    out-projection.
"""

import os
import sys

import numpy as np

for _p in ("/opt/trn_rl_repo",):
    if os.path.isdir(_p) and _p not in sys.path:
        sys.path.insert(0, _p)

os.environ.setdefault("MYCRO_LOCAL_CACHE", "1")


def _ensure_ntff_hook():
    """bass_utils' trace path imports antenv.axon_hooks; some images lack
    it. Recreate the module with the same ctypes hook if missing."""
    try:
        import antenv.axon_hooks  # noqa: F401

        return
    except ImportError:
        pass
    try:
        import types

        import antenv  # noqa: F401
        from trn_agent_boot.trn_boot import _ntff_profile_via_ctypes

        hook = _ntff_profile_via_ctypes("/opt/axon/libaxon_pjrt.so")
        mod = types.ModuleType("antenv.axon_hooks")
        mod.get_axon_ntff_profile_hook = lambda: hook
        mod.set_axon_ntff_profile_hook = lambda h: None
        sys.modules["antenv.axon_hooks"] = mod
    except Exception:
        pass


_ensure_ntff_hook()

import concourse.bass as bass  # noqa: E402
from concourse import bacc  # noqa: E402
import concourse.mybir as mybir  # noqa: E402
import concourse.tile as tile  # noqa: E402
from concourse.bass_utils import run_bass_kernel_spmd  # noqa: E402
from concourse.tile_rust import add_dep_helper  # noqa: E402

F32 = mybir.dt.float32
F32R = mybir.dt.float32r
BF16 = mybir.dt.bfloat16

B, T, C, H, DH = 2, 4096, 512, 8, 64
HEADS_PER_CORE = 2
HD = HEADS_PER_CORE * DH  # 128: head dims owned by one core
N_CORES = 8
QT_TILE = 512  # queries per attention tile
KC = 128  # keys per chunk (contraction granularity)
N_QT = T // QT_TILE  # 8
N_KC = T // KC  # 32
CK = C // 128  # 4 contraction chunks for the projections
SCALE = 1.0 / float(np.sqrt(DH))


def build_program():
    nc = bacc.Bacc(None)

    xT = nc.declare_dram_parameter("xT", [C, T], BF16, isOutput=False)
    wqT = nc.declare_dram_parameter("wqT", [C, HD], BF16, isOutput=False)
    wkT = nc.declare_dram_parameter("wkT", [C, HD], BF16, isOutput=False)
    wvT = nc.declare_dram_parameter("wvT", [C, HD], BF16, isOutput=False)
    # woT[p, j]: rows of w_out for this core's head dims; rows 0-63 = head0
    # dims, 64-127 = head1 dims (matches the stacked YTn layout, so the
    # out-projection is ONE K=128 matmul summing both heads).
    woT = nc.declare_dram_parameter("woT", [HD, C], BF16, isOutput=False)
    bq = nc.declare_dram_parameter("bq", [HD], F32, isOutput=False)
    bk = nc.declare_dram_parameter("bk", [HD], F32, isOutput=False)
    bv = nc.declare_dram_parameter("bv", [HD], F32, isOutput=False)
    out = nc.declare_dram_parameter("out", [T, C], F32, isOutput=True)

    with tile.TileContext(nc) as tc:
        with (
            tc.tile_pool(name="singles", bufs=1) as singles,
            tc.tile_pool(name="xin", bufs=8) as xin,
            tc.tile_pool(name="exps", bufs=4) as exps,
            tc.tile_pool(name="osb", bufs=3) as osb,
            tc.tile_pool(name="norm", bufs=2) as norm,
            tc.tile_pool(name="ps_proj", bufs=2, space="PSUM") as ps_proj,
            tc.tile_pool(name="ps_s", bufs=2, space="PSUM") as ps_s,
            tc.tile_pool(name="ps_yt", bufs=1, space="PSUM") as ps_yt,
        ):
            # ---- resident inputs (x0 + q/k weights first: they gate the
            # first PE work) --------------------------------------------
            xT_ap = xT.rearrange("(ko p) t -> p ko t", p=128)
            xt_first = xin.tile([128, CK, QT_TILE], BF16, tag="xt", name="xt_first")
            nc.sync.dma_start(xt_first, xT_ap[:, :, bass.ts(0, QT_TILE)])
            wqT_sb = singles.tile([128, CK, HD], BF16)
            nc.sync.dma_start(wqT_sb, wqT.rearrange("(ko p) m -> p ko m", p=128))
            wkT_sb = singles.tile([128, CK, HD], BF16)
            nc.sync.dma_start(wkT_sb, wkT.rearrange("(ko p) m -> p ko m", p=128))
            wvT_sb = singles.tile([128, CK, HD], BF16)
            nc.sync.dma_start(wvT_sb, wvT.rearrange("(ko p) m -> p ko m", p=128))
            woT_sb = singles.tile([HD, C], BF16)
            nc.sync.dma_start(woT_sb, woT[:])

            bq_col = singles.tile([128, 1], F32)
            nc.sync.dma_start(bq_col, bq.rearrange("(p one) -> p one", one=1))
            bk_col = singles.tile([128, 1], F32)
            nc.sync.dma_start(bk_col, bk.rearrange("(p one) -> p one", one=1))
            bv_row = singles.tile([1, HD], F32)
            nc.sync.dma_start(bv_row, bv[None, :])

            ones_f32 = singles.tile([128, 128], F32)
            nc.vector.memset(ones_f32, 1.0)
            ones_bf = singles.tile([128, 4], BF16)
            nc.vector.tensor_copy(ones_bf, ones_f32[:, 0:4])

            # [128,128] causal triangle: tri[k, qq] = 1 if k <= qq.
            # Built in F32 (affine_select crashes on other dtypes), then cast.
            tri_f32 = singles.tile([128, 128], F32)
            nc.vector.memset(tri_f32, 1.0)
            nc.gpsimd.affine_select(
                out=tri_f32,
                in_=tri_f32,
                compare_op=mybir.AluOpType.is_ge,
                fill=0.0,
                base=0,
                pattern=[[1, 128]],
                channel_multiplier=-1,
            )
            tri_bf = singles.tile([128, 128], BF16)
            nc.vector.tensor_copy(tri_bf, tri_f32)

            # broadcast bv across partitions on gpsimd
            bias_v_sb = singles.tile([128, HD], F32)
            nc.gpsimd.partition_broadcast(bias_v_sb, bv_row)
            bias_v2 = bias_v_sb.rearrange("p (h x) -> p h x", h=2)

            # per-tile storage (separate tile objects -> precise deps)
            QT_t = [
                singles.tile([128, QT_TILE], BF16, name=f"qtt{i}", tag=f"qtt{i}")
                for i in range(N_QT)
            ]
            KT_t = [
                singles.tile([128, QT_TILE], BF16, name=f"ktt{i}", tag=f"ktt{i}")
                for i in range(N_QT)
            ]
            # V chunks in [k, d] layout; per tile: 4 chunks of
            # [V0 | ones | V1 | ones] (65-column stride per head slice)
            V_t = [
                singles.tile([128, 4, 130], BF16, name=f"vt{i}", tag=f"vt{i}")
                for i in range(N_QT)
            ]
            # unnormalized attention outputs + denominators (deferred norm)
            YTu_t = [
                [
                    singles.tile(
                        [64, QT_TILE], BF16, name=f"ytu{h}_{i}", tag=f"ytu{h}_{i}"
                    )
                    for i in range(N_QT)
                ]
                for h in range(2)
            ]
            den_t = [
                [
                    singles.tile([1, QT_TILE], F32, name=f"den{h}_{i}", tag=f"den{h}_{i}")
                    for i in range(N_QT)
                ]
                for h in range(2)
            ]
            # normalized YT, both heads stacked on partitions (h0: 0-63,
            # h1: 64-127) so the out-projection contracts K=128 in one shot
            YTn_t = [
                singles.tile([128, QT_TILE], BF16, name=f"ytn{i}", tag=f"ytn{i}")
                for i in range(N_QT)
            ]
            for i in range(N_QT):
                nc.vector.tensor_copy(V_t[i][:, :, 64:65], ones_bf[:, :, None])
                nc.vector.tensor_copy(V_t[i][:, :, 129:130], ones_bf[:, :, None])

            def emit_qproj(qt, xt):
                ps_q = ps_proj.tile([128, QT_TILE], F32, tag="psproj", name="ps_q")
                for kc in range(CK):
                    nc.tensor.matmul(
                        ps_q,
                        wqT_sb[:, kc, :],
                        xt[:, kc, :],
                        start=(kc == 0),
                        stop=(kc == CK - 1),
                    )
                nc.vector.tensor_scalar_add(QT_t[qt][:], ps_q, bq_col)

            def emit_kproj(qt, xt):
                ps_k = ps_proj.tile([128, QT_TILE], F32, tag="psproj", name="ps_k")
                for kc in range(CK):
                    nc.tensor.matmul(
                        ps_k,
                        wkT_sb[:, kc, :],
                        xt[:, kc, :],
                        start=(kc == 0),
                        stop=(kc == CK - 1),
                    )
                nc.vector.tensor_scalar_add(KT_t[qt][:], ps_k, bk_col)

            def emit_vproj(qt, xt, sv):
                ps_v = ps_proj.tile([128, HD], F32, tag="psproj", name="ps_v")
                for kc in range(CK):
                    nc.tensor.matmul(
                        ps_v,
                        xt[:, kc, bass.ts(sv, 128)],
                        wvT_sb[:, kc, :],
                        start=(kc == 0),
                        stop=(kc == CK - 1),
                    )
                vt = V_t[qt]
                v_vals = bass.AP(
                    tensor=vt.tensor,
                    offset=vt.offset,
                    ap=[vt.ap[0], vt.ap[1], [65, 2], [1, 64]],
                )
                nc.vector.tensor_add(
                    v_vals[:, sv],
                    ps_v.rearrange("p (h x) -> p h x", h=2),
                    bias_v2,
                )

            def emit_norm_a(qt, yt_ps):
                # evict yt_ps fast (den row + unnormalized YT), then
                # broadcast den across partitions on the idle gpsimd so
                # neither PE nor the DVE queue head ever waits on it.
                for h in range(2):
                    nc.vector.tensor_copy(den_t[h][qt][:], yt_ps[h][64:65, :])
                for h in range(2):
                    nc.vector.tensor_copy(YTu_t[h][qt], yt_ps[h][0:64, :])
                den_bc = []
                for h in range(2):
                    bc = norm.tile(
                        [64, QT_TILE], F32, tag=f"denbc{h}", name=f"denbc{h}"
                    )
                    nc.gpsimd.partition_broadcast(bc, den_t[h][qt][:])
                    den_bc.append(bc)
                return den_bc

            def emit_norm_b(qt, den_bc):
                # ~51-ULP reciprocal (5x faster than the iterative divide),
                # then scale the unnormalized attention rows.
                for h in range(2):
                    rec_sb = norm.tile(
                        [64, QT_TILE], F32, tag=f"rec{h}", name=f"rec{h}"
                    )
                    nc.vector.reciprocal_approx_fast(rec_sb, den_bc[h])
                    nc.vector.tensor_mul(
                        YTn_t[qt][64 * h : 64 * h + 64, :],
                        YTu_t[h][qt][:],
                        rec_sb,
                    )

            def emit_outproj_sv(qt, sv):
                tc8 = qt * (QT_TILE // 128) + sv
                ps_o = ps_proj.tile([128, C], F32, tag="psproj", name="ps_o")
                nc.tensor.matmul(
                    ps_o,
                    YTn_t[qt][:, bass.ts(sv, 128)],
                    woT_sb,
                    start=True,
                    stop=True,
                )
                o_sb = osb.tile([128, C], F32, tag="osb")
                nc.vector.tensor_copy(o_sb, ps_o)
                nc.sync.dma_start(out[bass.ts(tc8, 128), :], o_sb)

            xt_tiles = {0: xt_first}

            def emit_xt(i):
                if i not in xt_tiles and i < N_QT:
                    xt_i = xin.tile(
                        [128, CK, QT_TILE], BF16, tag="xt", name=f"xt{i}"
                    )
                    nc.sync.dma_start(xt_i, xT_ap[:, :, bass.ts(i, QT_TILE)])
                    xt_tiles[i] = xt_i

            def emit_s_exp(qt2, pair):
                """S^T quad (both heads, 2 chunks) + exp + causal mask for
                (query tile qt2, chunk pair). Diagonal chunks skip the
                fully-masked 128r query prefix (excluded from Y, never
                read) and mask only the [128,128] triangle block."""
                s_ps = [
                    ps_s.tile([128, 2, QT_TILE], F32, tag="s", name=f"s{h}")
                    for h in range(2)
                ]
                prev_mm = None
                for sub in range(2):
                    c = pair * 2 + sub
                    r = c - 4 * qt2
                    off = KC * r if r > 0 else 0
                    for h in range(2):
                        hp = slice(h * 64, h * 64 + 64)
                        mm = nc.tensor.matmul(
                            s_ps[h][:, sub, off:],
                            KT_t[c // 4][hp, bass.ts(c % 4, KC)],
                            QT_t[qt2][hp, off:],
                            start=True,
                            stop=True,
                        )
                        # keep the quad h-alternating so adjacent matmuls
                        # land on different PE row groups
                        if prev_mm is not None:
                            add_dep_helper(
                                mm.ins,
                                prev_mm.ins,
                                sync=False,
                                reason="s-quad row-group alternation",
                            )
                        prev_mm = mm
                e_sb = exps.tile([128, 2, 2, QT_TILE], BF16, tag="e", name="e")
                for h in range(2):
                    nc.scalar.activation(
                        e_sb[:, h],
                        s_ps[h],
                        mybir.ActivationFunctionType.Exp,
                        scale=SCALE,
                    )
                for sub in range(2):
                    c = pair * 2 + sub
                    r = c - 4 * qt2
                    if r >= 0:
                        for h in range(2):
                            nc.vector.tensor_mul(
                                e_sb[:, h, sub, bass.ts(r, KC)],
                                e_sb[:, h, sub, bass.ts(r, KC)],
                                tri_bf,
                            )
                return e_sb

            qproj_done = set()
            den_bcs = {}
            pending_e = {}
            for i in range(1, N_QT):
                emit_xt(i)
            for qt in range(N_QT):
                xt = xt_tiles[qt]
                if qt not in qproj_done:
                    emit_qproj(qt, xt)
                    qproj_done.add(qt)
                if qt == 0:
                    emit_kproj(qt, xt)
                    for sv in range(4):
                        emit_vproj(qt, xt, sv)

                yt_ps = [
                    ps_yt.tile([128, QT_TILE], F32, tag=f"yt{h}", name=f"yt{h}")
                    for h in range(2)
                ]
                n_pairs = 2 * (qt + 1)
                outproj_at = {}
                for sv in range(4):
                    outproj_at.setdefault(min(3 + sv, n_pairs - 1), []).append(sv)
                for pair in range(n_pairs):
                    e_sb = pending_e.pop((qt, pair), None)
                    if e_sb is None:
                        e_sb = emit_s_exp(qt, pair)
                    # pipelined projections / out-proj for other tiles
                    if pair == 0 and qt > 0:
                        emit_kproj(qt, xt)
                    if qt > 0 and pair < 4:
                        emit_vproj(qt, xt, pair)
                    if pair == min(2, n_pairs - 1) and qt + 1 < N_QT:
                        emit_qproj(qt + 1, xt_tiles[qt + 1])
                        qproj_done.add(qt + 1)
                    if pair == n_pairs - 1 and qt + 1 < N_QT:
                        # hoist the next tile's first S quad + exp so PE/ACT
                        # never idle across the tile boundary
                        pending_e[(qt + 1, 0)] = emit_s_exp(qt + 1, 0)
                    for h in range(2):
                        for sub in range(2):
                            c = pair * 2 + sub
                            r = c - 4 * qt
                            off = KC * r if r > 0 else 0
                            nc.tensor.matmul(
                                yt_ps[h][0:65, off:],
                                V_t[c // 4][:, c % 4, h * 65 : h * 65 + 65],
                                e_sb[:, h, sub, off:],
                                start=(pair == 0 and sub == 0),
                                stop=(pair == n_pairs - 1 and sub == 1),
                            )
                    if pair == 1 and qt > 0:
                        emit_norm_b(qt - 1, den_bcs[qt - 1])
                    if qt > 0:
                        for sv in outproj_at.get(pair, []):
                            emit_outproj_sv(qt - 1, sv)

                # ---- evict yt_ps fast + deferred normalization ----
                den_bcs[qt] = emit_norm_a(qt, yt_ps)
            emit_norm_b(N_QT - 1, den_bcs[N_QT - 1])
            for sv in range(4):
                emit_outproj_sv(N_QT - 1, sv)

    return nc


_PROGRAM = None


def _get_program():
    global _PROGRAM
    if _PROGRAM is None:
        _PROGRAM = build_program()
        if not _PROGRAM.is_finalized():
            _PROGRAM.finalize()
    return _PROGRAM


def make_in_maps(x, w_qkv, b_qkv, w_out, b_out):
    """Shard the full inputs into per-core input maps."""
    import ml_dtypes

    bf16 = ml_dtypes.bfloat16
    x = np.ascontiguousarray(x, dtype=np.float32)
    w_qkv = np.ascontiguousarray(w_qkv, dtype=np.float32)
    b_qkv = np.ascontiguousarray(b_qkv, dtype=np.float32)
    w_out = np.ascontiguousarray(w_out, dtype=np.float32)

    wq = w_qkv[0:C]  # [C, C] rows = q features
    wk = w_qkv[C : 2 * C]
    wv = w_qkv[2 * C : 3 * C]
    bq_full = b_qkv[0:C]
    bk_full = b_qkv[C : 2 * C]
    bv_full = b_qkv[2 * C : 3 * C]

    xT_b = [np.ascontiguousarray(x[b].T.astype(bf16)) for b in range(B)]

    in_maps = []
    for core in range(N_CORES):
        b = core // 4
        g = core % 4
        rows = slice(g * HD, (g + 1) * HD)  # this core's head dims
        woT = np.ascontiguousarray(w_out[:, rows].T.astype(bf16))  # [HD, C]
        in_maps.append(
            {
                "xT": xT_b[b],
                "wqT": np.ascontiguousarray(wq[rows].T.astype(bf16)),
                "wkT": np.ascontiguousarray(wk[rows].T.astype(bf16)),
                "wvT": np.ascontiguousarray(wv[rows].T.astype(bf16)),
                "woT": woT,
                "bq": np.ascontiguousarray(bq_full[rows]),
                "bk": np.ascontiguousarray(bk_full[rows]),
                "bv": np.ascontiguousarray(bv_full[rows]),
            }
        )
    return in_maps


def kernel(x, w_qkv, b_qkv, w_out, b_out, _trace=False, _trace_kwargs=None):
    in_maps = make_in_maps(x, w_qkv, b_qkv, w_out, b_out)
    nc = _get_program()
    res = run_bass_kernel_spmd(
        nc,
        in_maps,
        list(range(N_CORES)),
        trace=_trace,
        **(_trace_kwargs or {}),
    )
    outs = [res.results[c]["out"] for c in range(N_CORES)]
    bo = np.asarray(b_out, dtype=np.float32)
    # unshard: sum the 4 row-parallel partials per batch (+ bias), stack
    y = np.stack(
        [
            outs[0] + outs[1] + outs[2] + outs[3] + bo,
            outs[4] + outs[5] + outs[6] + outs[7] + bo,
        ]
    ).astype(np.float32)
    if _trace:
        return y, res
    return y


# revision 38
# speedup vs baseline: 1.2642x; 1.2642x over previous
"""Causal self-attention (B=2, T=4096, C=512, H=8, Dh=64) on 8 trn2 cores.

Sharding: core = (batch, head-pair). 2 batches x 4 head-pairs = 8 cores.
Each core computes q/k/v projections for its 2 heads, causal attention in
S^T ([k, q]) layout, and a row-parallel slice of the output projection.
Host sums the 4 partial outputs per batch (+ b_out) and stacks batches.

bf16 pipeline (PSUM accumulation stays f32 where it matters):
  - x / weights arrive bf16; Q/K/V produced bf16 (projection matmuls
    accumulate f32 in PSUM, DVE bias-add casts to bf16).
  - S^T = KT-chunk.T @ QT written to a bf16 PSUM tile [128, 2, 2, 512]
    covering BOTH heads of a chunk pair -> ONE exp activation per pair
    at [128, 2048] (amortizes ACT's +352cyc/instr overhead).
  - Causal mask: only the [128,128] triangle block of each diagonal
    chunk is multiplied (DVE bf16); the fully-masked 128r-column prefix
    is excluded by narrowing the Y matmul instead of zeroing.
  - YT[h][65, 512] += V_chunk @ expS in f32 PSUM (row 64 = softmax
    denominator via an appended ones column in V).
  - Deferred normalization: yt_ps evicted immediately (bf16 YTu + f32r
    den row) so the next tile's Y matmuls never wait on the reciprocal;
    recip -> PE partition-broadcast -> DVE multiply runs one tile behind,
    interleaved with the next tile's attention, as does the row-parallel
    out-projection.
"""

import os
import sys

import numpy as np

for _p in ("/opt/trn_rl_repo",):
    if os.path.isdir(_p) and _p not in sys.path:
        sys.path.insert(0, _p)

os.environ.setdefault("MYCRO_LOCAL_CACHE", "1")


def _ensure_ntff_hook():
    """bass_utils' trace path imports antenv.axon_hooks; some images lack
    it. Recreate the module with the same ctypes hook if missing."""
    try:
        import antenv.axon_hooks  # noqa: F401

        return
    except ImportError:
        pass
    try:
        import types

        import antenv  # noqa: F401
        from trn_agent_boot.trn_boot import _ntff_profile_via_ctypes

        hook = _ntff_profile_via_ctypes("/opt/axon/libaxon_pjrt.so")
        mod = types.ModuleType("antenv.axon_hooks")
        mod.get_axon_ntff_profile_hook = lambda: hook
        mod.set_axon_ntff_profile_hook = lambda h: None
        sys.modules["antenv.axon_hooks"] = mod
    except Exception:
        pass


_ensure_ntff_hook()

import concourse.bass as bass  # noqa: E402
from concourse import bacc  # noqa: E402
import concourse.mybir as mybir  # noqa: E402
import concourse.tile as tile  # noqa: E402
from concourse.bass_utils import run_bass_kernel_spmd  # noqa: E402
from concourse.tile_rust import add_dep_helper  # noqa: E402

F32 = mybir.dt.float32
F32R = mybir.dt.float32r
BF16 = mybir.dt.bfloat16

B, T, C, H, DH = 2, 4096, 512, 8, 64
HEADS_PER_CORE = 2
HD = HEADS_PER_CORE * DH  # 128: head dims owned by one core
N_CORES = 8
QT_TILE = 512  # queries per attention tile
KC = 128  # keys per chunk (contraction granularity)
N_QT = T // QT_TILE  # 8
N_KC = T // KC  # 32
CK = C // 128  # 4 contraction chunks for the projections
SCALE = 1.0 / float(np.sqrt(DH))


def build_program():
    nc = bacc.Bacc(None)

    xT = nc.declare_dram_parameter("xT", [C, T], BF16, isOutput=False)
    wqT = nc.declare_dram_parameter("wqT", [C, HD], BF16, isOutput=False)
    wkT = nc.declare_dram_parameter("wkT", [C, HD], BF16, isOutput=False)
    wvT = nc.declare_dram_parameter("wvT", [C, HD], BF16, isOutput=False)
    # woT[p, j]: rows of w_out for this core's head dims; rows 0-63 = head0
    # dims, 64-127 = head1 dims (matches the stacked YTn layout, so the
    # out-projection is ONE K=128 matmul summing both heads).
    woT = nc.declare_dram_parameter("woT", [HD, C], BF16, isOutput=False)
    bq = nc.declare_dram_parameter("bq", [HD], F32, isOutput=False)
    bk = nc.declare_dram_parameter("bk", [HD], F32, isOutput=False)
    bv = nc.declare_dram_parameter("bv", [HD], F32, isOutput=False)
    out = nc.declare_dram_parameter("out", [T, C], F32, isOutput=True)

    with tile.TileContext(nc) as tc:
        with (
            tc.tile_pool(name="singles", bufs=1) as singles,
            tc.tile_pool(name="xin", bufs=8) as xin,
            tc.tile_pool(name="exps", bufs=4) as exps,
            tc.tile_pool(name="osb", bufs=3) as osb,
            tc.tile_pool(name="norm", bufs=2) as norm,
            tc.tile_pool(name="ps_proj", bufs=2, space="PSUM") as ps_proj,
            tc.tile_pool(name="ps_s", bufs=2, space="PSUM") as ps_s,
            tc.tile_pool(name="ps_yt", bufs=1, space="PSUM") as ps_yt,
        ):
            # ---- resident inputs (x0 + q/k weights first: they gate the
            # first PE work) --------------------------------------------
            xT_ap = xT.rearrange("(ko p) t -> p ko t", p=128)
            xt_first = xin.tile([128, CK, QT_TILE], BF16, tag="xt", name="xt_first")
            nc.sync.dma_start(xt_first, xT_ap[:, :, bass.ts(0, QT_TILE)])
            wqT_sb = singles.tile([128, CK, HD], BF16)
            nc.sync.dma_start(wqT_sb, wqT.rearrange("(ko p) m -> p ko m", p=128))
            wkT_sb = singles.tile([128, CK, HD], BF16)
            nc.sync.dma_start(wkT_sb, wkT.rearrange("(ko p) m -> p ko m", p=128))
            wvT_sb = singles.tile([128, CK, HD], BF16)
            nc.sync.dma_start(wvT_sb, wvT.rearrange("(ko p) m -> p ko m", p=128))
            woT_sb = singles.tile([HD, C], BF16)
            nc.sync.dma_start(woT_sb, woT[:])

            bq_col = singles.tile([128, 1], F32)
            nc.sync.dma_start(bq_col, bq.rearrange("(p one) -> p one", one=1))
            bk_col = singles.tile([128, 1], F32)
            nc.sync.dma_start(bk_col, bk.rearrange("(p one) -> p one", one=1))
            bv_row = singles.tile([1, HD], F32)
            nc.sync.dma_start(bv_row, bv[None, :])

            ones_f32 = singles.tile([128, 128], F32)
            nc.vector.memset(ones_f32, 1.0)
            ones_bf = singles.tile([128, 4], BF16)
            nc.vector.tensor_copy(ones_bf, ones_f32[:, 0:4])

            # [128,128] causal triangle: tri[k, qq] = 1 if k <= qq.
            # Built in F32 (affine_select crashes on other dtypes), then cast.
            tri_f32 = singles.tile([128, 128], F32)
            nc.vector.memset(tri_f32, 1.0)
            nc.gpsimd.affine_select(
                out=tri_f32,
                in_=tri_f32,
                compare_op=mybir.AluOpType.is_ge,
                fill=0.0,
                base=0,
                pattern=[[1, 128]],
                channel_multiplier=-1,
            )
            tri_bf = singles.tile([128, 128], BF16)
            nc.vector.tensor_copy(tri_bf, tri_f32)

            # broadcast bv across partitions on gpsimd
            bias_v_sb = singles.tile([128, HD], F32)
            nc.gpsimd.partition_broadcast(bias_v_sb, bv_row)
            bias_v2 = bias_v_sb.rearrange("p (h x) -> p h x", h=2)

            # per-tile storage (separate tile objects -> precise deps)
            QT_t = [
                singles.tile([128, QT_TILE], BF16, name=f"qtt{i}", tag=f"qtt{i}")
                for i in range(N_QT)
            ]
            KT_t = [
                singles.tile([128, QT_TILE], BF16, name=f"ktt{i}", tag=f"ktt{i}")
                for i in range(N_QT)
            ]
            # V chunks in [k, d] layout; per tile: 4 chunks of
            # [V0 | ones | V1 | ones] (65-column stride per head slice)
            V_t = [
                singles.tile([128, 4, 130], BF16, name=f"vt{i}", tag=f"vt{i}")
                for i in range(N_QT)
            ]
            # unnormalized attention outputs + denominators (deferred norm)
            YTu_t = [
                [
                    singles.tile(
                        [64, QT_TILE], BF16, name=f"ytu{h}_{i}", tag=f"ytu{h}_{i}"
                    )
                    for i in range(N_QT)
                ]
                for h in range(2)
            ]
            den_t = [
                [
                    singles.tile([1, QT_TILE], F32, name=f"den{h}_{i}", tag=f"den{h}_{i}")
                    for i in range(N_QT)
                ]
                for h in range(2)
            ]
            # normalized YT, both heads stacked on partitions (h0: 0-63,
            # h1: 64-127) so the out-projection contracts K=128 in one shot
            YTn_t = [
                singles.tile([128, QT_TILE], BF16, name=f"ytn{i}", tag=f"ytn{i}")
                for i in range(N_QT)
            ]
            for i in range(N_QT):
                nc.vector.tensor_copy(V_t[i][:, :, 64:65], ones_bf[:, :, None])
                nc.vector.tensor_copy(V_t[i][:, :, 129:130], ones_bf[:, :, None])

            def emit_qproj(qt, xt):
                ps_q = ps_proj.tile([128, QT_TILE], F32, tag="psproj", name="ps_q")
                for kc in range(CK):
                    nc.tensor.matmul(
                        ps_q,
                        wqT_sb[:, kc, :],
                        xt[:, kc, :],
                        start=(kc == 0),
                        stop=(kc == CK - 1),
                    )
                nc.vector.tensor_scalar_add(QT_t[qt][:], ps_q, bq_col)

            def emit_kproj(qt, xt):
                ps_k = ps_proj.tile([128, QT_TILE], F32, tag="psproj", name="ps_k")
                for kc in range(CK):
                    nc.tensor.matmul(
                        ps_k,
                        wkT_sb[:, kc, :],
                        xt[:, kc, :],
                        start=(kc == 0),
                        stop=(kc == CK - 1),
                    )
                nc.vector.tensor_scalar_add(KT_t[qt][:], ps_k, bk_col)

            def emit_vproj(qt, xt, sv):
                ps_v = ps_proj.tile([128, HD], F32, tag="psproj", name="ps_v")
                for kc in range(CK):
                    nc.tensor.matmul(
                        ps_v,
                        xt[:, kc, bass.ts(sv, 128)],
                        wvT_sb[:, kc, :],
                        start=(kc == 0),
                        stop=(kc == CK - 1),
                    )
                vt = V_t[qt]
                v_vals = bass.AP(
                    tensor=vt.tensor,
                    offset=vt.offset,
                    ap=[vt.ap[0], vt.ap[1], [65, 2], [1, 64]],
                )
                nc.vector.tensor_add(
                    v_vals[:, sv],
                    ps_v.rearrange("p (h x) -> p h x", h=2),
                    bias_v2,
                )

            def emit_norm_a(qt, yt_ps):
                # evict yt_ps fast (den row + unnormalized YT), then
                # broadcast den across partitions on the idle gpsimd so
                # neither PE nor the DVE queue head ever waits on it.
                for h in range(2):
                    nc.vector.tensor_copy(den_t[h][qt][:], yt_ps[h][64:65, :])
                for h in range(2):
                    nc.vector.tensor_copy(YTu_t[h][qt], yt_ps[h][0:64, :])
                den_bc = []
                for h in range(2):
                    bc = norm.tile(
                        [64, QT_TILE], F32, tag=f"denbc{h}", name=f"denbc{h}"
                    )
                    nc.gpsimd.partition_broadcast(bc, den_t[h][qt][:])
                    den_bc.append(bc)
                return den_bc

            def emit_norm_b(qt, den_bc):
                # ~51-ULP reciprocal (5x faster than the iterative divide),
                # then scale the unnormalized attention rows.
                for h in range(2):
                    rec_sb = norm.tile(
                        [64, QT_TILE], F32, tag=f"rec{h}", name=f"rec{h}"
                    )
                    nc.vector.reciprocal_approx_fast(rec_sb, den_bc[h])
                    nc.vector.tensor_mul(
                        YTn_t[qt][64 * h : 64 * h + 64, :],
                        YTu_t[h][qt][:],
                        rec_sb,
                    )

            def emit_outproj_sv(qt, sv):
                tc8 = qt * (QT_TILE // 128) + sv
                ps_o = ps_proj.tile([128, C], F32, tag="psproj", name="ps_o")
                nc.tensor.matmul(
                    ps_o,
                    YTn_t[qt][:, bass.ts(sv, 128)],
                    woT_sb,
                    start=True,
                    stop=True,
                )
                o_sb = osb.tile([128, C], F32, tag="osb")
                nc.vector.tensor_copy(o_sb, ps_o)
                nc.sync.dma_start(out[bass.ts(tc8, 128), :], o_sb)

            xt_tiles = {0: xt_first}

            def emit_xt(i):
                if i not in xt_tiles and i < N_QT:
                    xt_i = xin.tile(
                        [128, CK, QT_TILE], BF16, tag="xt", name=f"xt{i}"
                    )
                    nc.sync.dma_start(xt_i, xT_ap[:, :, bass.ts(i, QT_TILE)])
                    xt_tiles[i] = xt_i

            def emit_s_exp(qt2, pair):
                """S^T quad (both heads, 2 chunks) + exp + causal mask for
                (query tile qt2, chunk pair). Diagonal chunks skip the
                fully-masked 128r query prefix (excluded from Y, never
                read) and mask only the [128,128] triangle block."""
                s_ps = [
                    ps_s.tile([128, 2, QT_TILE], F32, tag="s", name=f"s{h}")
                    for h in range(2)
                ]
                for sub in range(2):
                    c = pair * 2 + sub
                    r = c - 4 * qt2
                    off = KC * r if r > 0 else 0
                    for h in range(2):
                        hp = slice(h * 64, h * 64 + 64)
                        nc.tensor.matmul(
                            s_ps[h][:, sub, off:],
                            KT_t[c // 4][hp, bass.ts(c % 4, KC)],
                            QT_t[qt2][hp, off:],
                            start=True,
                            stop=True,
                        )
                e_sb = exps.tile([128, 2, 2, QT_TILE], BF16, tag="e", name="e")
                for h in range(2):
                    nc.scalar.activation(
                        e_sb[:, h],
                        s_ps[h],
                        mybir.ActivationFunctionType.Exp,
                        scale=SCALE,
                    )
                for sub in range(2):
                    c = pair * 2 + sub
                    r = c - 4 * qt2
                    if r >= 0:
                        for h in range(2):
                            nc.vector.tensor_mul(
                                e_sb[:, h, sub, bass.ts(r, KC)],
                                e_sb[:, h, sub, bass.ts(r, KC)],
                                tri_bf,
                            )
                return e_sb

            qproj_done = set()
            den_bcs = {}
            pending_e = {}
            for i in range(1, N_QT):
                emit_xt(i)
            for qt in range(N_QT):
                xt = xt_tiles[qt]
                if qt not in qproj_done:
                    emit_qproj(qt, xt)
                    qproj_done.add(qt)
                if qt == 0:
                    emit_kproj(qt, xt)
                    for sv in range(4):
                        emit_vproj(qt, xt, sv)

                yt_ps = [
                    ps_yt.tile([128, QT_TILE], F32, tag=f"yt{h}", name=f"yt{h}")
                    for h in range(2)
                ]
                n_pairs = 2 * (qt + 1)
                outproj_at = {}
                for sv in range(4):
                    outproj_at.setdefault(min(3 + sv, n_pairs - 1), []).append(sv)
                for pair in range(n_pairs):
                    e_sb = pending_e.pop((qt, pair), None)
                    if e_sb is None:
                        e_sb = emit_s_exp(qt, pair)
                    # pipelined projections / out-proj for other tiles
                    if pair == 0 and qt > 0:
                        emit_kproj(qt, xt)
                    if qt > 0 and pair < 4:
                        emit_vproj(qt, xt, pair)
                    if pair == min(2, n_pairs - 1) and qt + 1 < N_QT:
                        emit_qproj(qt + 1, xt_tiles[qt + 1])
                        qproj_done.add(qt + 1)
                    if pair == n_pairs - 1 and qt + 1 < N_QT:
                        # hoist the next tile's first S quad + exp so PE/ACT
                        # never idle across the tile boundary
                        pending_e[(qt + 1, 0)] = emit_s_exp(qt + 1, 0)
                    for h in range(2):
                        for sub in range(2):
                            c = pair * 2 + sub
                            r = c - 4 * qt
                            off = KC * r if r > 0 else 0
                            nc.tensor.matmul(
                                yt_ps[h][0:65, off:],
                                V_t[c // 4][:, c % 4, h * 65 : h * 65 + 65],
                                e_sb[:, h, sub, off:],
                                start=(pair == 0 and sub == 0),
                                stop=(pair == n_pairs - 1 and sub == 1),
                            )
                    if pair == 1 and qt > 0:
                        emit_norm_b(qt - 1, den_bcs[qt - 1])
                    if qt > 0:
                        for sv in outproj_at.get(pair, []):
                            emit_outproj_sv(qt - 1, sv)

                # ---- evict yt_ps fast + deferred normalization ----
                den_bcs[qt] = emit_norm_a(qt, yt_ps)
            emit_norm_b(N_QT - 1, den_bcs[N_QT - 1])
            for sv in range(4):
                emit_outproj_sv(N_QT - 1, sv)

    return nc


_PROGRAM = None


def _get_program():
    global _PROGRAM
    if _PROGRAM is None:
        _PROGRAM = build_program()
        if not _PROGRAM.is_finalized():
            _PROGRAM.finalize()
    return _PROGRAM


def make_in_maps(x, w_qkv, b_qkv, w_out, b_out):
    """Shard the full inputs into per-core input maps."""
    import ml_dtypes

    bf16 = ml_dtypes.bfloat16
    x = np.ascontiguousarray(x, dtype=np.float32)
    w_qkv = np.ascontiguousarray(w_qkv, dtype=np.float32)
    b_qkv = np.ascontiguousarray(b_qkv, dtype=np.float32)
    w_out = np.ascontiguousarray(w_out, dtype=np.float32)

    wq = w_qkv[0:C]  # [C, C] rows = q features
    wk = w_qkv[C : 2 * C]
    wv = w_qkv[2 * C : 3 * C]
    bq_full = b_qkv[0:C]
    bk_full = b_qkv[C : 2 * C]
    bv_full = b_qkv[2 * C : 3 * C]

    xT_b = [np.ascontiguousarray(x[b].T.astype(bf16)) for b in range(B)]

    in_maps = []
    for core in range(N_CORES):
        b = core // 4
        g = core % 4
        rows = slice(g * HD, (g + 1) * HD)  # this core's head dims
        woT = np.ascontiguousarray(w_out[:, rows].T.astype(bf16))  # [HD, C]
        in_maps.append(
            {
                "xT": xT_b[b],
                "wqT": np.ascontiguousarray(wq[rows].T.astype(bf16)),
                "wkT": np.ascontiguousarray(wk[rows].T.astype(bf16)),
                "wvT": np.ascontiguousarray(wv[rows].T.astype(bf16)),
                "woT": woT,
                "bq": np.ascontiguousarray(bq_full[rows]),
                "bk": np.ascontiguousarray(bk_full[rows]),
                "bv": np.ascontiguousarray(bv_full[rows]),
            }
        )
    return in_maps


def kernel(x, w_qkv, b_qkv, w_out, b_out, _trace=False, _trace_kwargs=None):
    in_maps = make_in_maps(x, w_qkv, b_qkv, w_out, b_out)
    nc = _get_program()
    res = run_bass_kernel_spmd(
        nc,
        in_maps,
        list(range(N_CORES)),
        trace=_trace,
        **(_trace_kwargs or {}),
    )
    outs = [res.results[c]["out"] for c in range(N_CORES)]
    bo = np.asarray(b_out, dtype=np.float32)
    # unshard: sum the 4 row-parallel partials per batch (+ bias), stack
    y = np.stack(
        [
            outs[0] + outs[1] + outs[2] + outs[3] + bo,
            outs[4] + outs[5] + outs[6] + outs[7] + bo,
        ]
    ).astype(np.float32)
    if _trace:
        return y, res
    return y


# revision 50
# speedup vs baseline: 1.2678x; 1.0029x over previous
"""Causal self-attention (B=2, T=4096, C=512, H=8, Dh=64) on 8 trn2 cores.

Sharding: core = (batch, head-pair). 2 batches x 4 head-pairs = 8 cores.
Each core computes q/k/v projections for its 2 heads, causal attention in
S^T ([k, q]) layout, and a row-parallel slice of the output projection.
Host sums the 4 partial outputs per batch (+ b_out) and stacks batches.

bf16 pipeline (PSUM accumulation stays f32 where it matters):
  - x / weights arrive bf16; Q/K/V produced bf16 (projection matmuls
    accumulate f32 in PSUM, DVE bias-add casts to bf16).
  - S^T = KT-chunk.T @ QT written to a bf16 PSUM tile [128, 2, 2, 512]
    covering BOTH heads of a chunk pair -> ONE exp activation per pair
    at [128, 2048] (amortizes ACT's +352cyc/instr overhead).
  - Causal mask: only the [128,128] triangle block of each diagonal
    chunk is multiplied (DVE bf16); the fully-masked 128r-column prefix
    is excluded by narrowing the Y matmul instead of zeroing.
  - YT[h][65, 512] += V_chunk @ expS in f32 PSUM (row 64 = softmax
    denominator via an appended ones column in V).
  - Deferred normalization: yt_ps evicted immediately (bf16 YTu + f32r
    den row) so the next tile's Y matmuls never wait on the reciprocal;
    recip -> PE partition-broadcast -> DVE multiply runs one tile behind,
    interleaved with the next tile's attention, as does the row-parallel
    out-projection.
"""

import os
import sys

import numpy as np

for _p in ("/opt/trn_rl_repo",):
    if os.path.isdir(_p) and _p not in sys.path:
        sys.path.insert(0, _p)

os.environ.setdefault("MYCRO_LOCAL_CACHE", "1")


def _ensure_ntff_hook():
    """bass_utils' trace path imports antenv.axon_hooks; some images lack
    it. Recreate the module with the same ctypes hook if missing."""
    try:
        import antenv.axon_hooks  # noqa: F401

        return
    except ImportError:
        pass
    try:
        import types

        import antenv  # noqa: F401
        from trn_agent_boot.trn_boot import _ntff_profile_via_ctypes

        hook = _ntff_profile_via_ctypes("/opt/axon/libaxon_pjrt.so")
        mod = types.ModuleType("antenv.axon_hooks")
        mod.get_axon_ntff_profile_hook = lambda: hook
        mod.set_axon_ntff_profile_hook = lambda h: None
        sys.modules["antenv.axon_hooks"] = mod
    except Exception:
        pass


_ensure_ntff_hook()

import concourse.bass as bass  # noqa: E402
from concourse import bacc  # noqa: E402
import concourse.mybir as mybir  # noqa: E402
import concourse.tile as tile  # noqa: E402
from concourse.bass_utils import run_bass_kernel_spmd  # noqa: E402
from concourse.tile_rust import add_dep_helper  # noqa: E402

F32 = mybir.dt.float32
F32R = mybir.dt.float32r
BF16 = mybir.dt.bfloat16
FP8 = mybir.dt.float8e4
MASK_NEG = -240.0  # exp(MASK_NEG * SCALE) == exp(-30) -> 0 in fp8

B, T, C, H, DH = 2, 4096, 512, 8, 64
HEADS_PER_CORE = 2
HD = HEADS_PER_CORE * DH  # 128: head dims owned by one core
N_CORES = 8
QT_TILE = 512  # queries per attention tile
KC = 128  # keys per chunk (contraction granularity)
N_QT = T // QT_TILE  # 8
N_KC = T // KC  # 32
CK = C // 128  # 4 contraction chunks for the projections
SCALE = 1.0 / float(np.sqrt(DH))


def build_program():
    nc = bacc.Bacc(None)

    xT = nc.declare_dram_parameter("xT", [C, T], BF16, isOutput=False)
    wqT = nc.declare_dram_parameter("wqT", [C, HD], BF16, isOutput=False)
    wkT = nc.declare_dram_parameter("wkT", [C, HD], BF16, isOutput=False)
    wvT = nc.declare_dram_parameter("wvT", [C, HD], BF16, isOutput=False)
    # woT[p, j]: rows of w_out for this core's head dims; rows 0-63 = head0
    # dims, 64-127 = head1 dims (matches the stacked YTn layout, so the
    # out-projection is ONE K=128 matmul summing both heads).
    woT = nc.declare_dram_parameter("woT", [HD, C], BF16, isOutput=False)
    bq = nc.declare_dram_parameter("bq", [HD], F32, isOutput=False)
    bk = nc.declare_dram_parameter("bk", [HD], F32, isOutput=False)
    bv = nc.declare_dram_parameter("bv", [HD], F32, isOutput=False)
    out = nc.declare_dram_parameter("out", [T, C], F32, isOutput=True)

    with tile.TileContext(nc) as tc:
        with (
            tc.tile_pool(name="singles", bufs=1) as singles,
            tc.tile_pool(name="xin", bufs=8) as xin,
            tc.tile_pool(name="exps", bufs=4) as exps,
            tc.tile_pool(name="osb", bufs=3) as osb,
            tc.tile_pool(name="norm", bufs=2) as norm,
            tc.tile_pool(name="ps_proj", bufs=2, space="PSUM") as ps_proj,
            tc.tile_pool(name="ps_s", bufs=2, space="PSUM") as ps_s,
            tc.tile_pool(name="ps_yt", bufs=1, space="PSUM") as ps_yt,
        ):
            # ---- resident inputs (x0 + q/k weights first: they gate the
            # first PE work) --------------------------------------------
            xT_ap = xT.rearrange("(ko p) t -> p ko t", p=128)
            xt_first = xin.tile([128, CK, QT_TILE], BF16, tag="xt", name="xt_first")
            nc.sync.dma_start(xt_first, xT_ap[:, :, bass.ts(0, QT_TILE)])
            wqT_sb = singles.tile([128, CK, HD], BF16)
            nc.sync.dma_start(wqT_sb, wqT.rearrange("(ko p) m -> p ko m", p=128))
            wkT_sb = singles.tile([128, CK, HD], BF16)
            nc.sync.dma_start(wkT_sb, wkT.rearrange("(ko p) m -> p ko m", p=128))
            wvT_sb = singles.tile([128, CK, HD], BF16)
            nc.sync.dma_start(wvT_sb, wvT.rearrange("(ko p) m -> p ko m", p=128))
            woT_sb = singles.tile([HD, C], BF16)
            nc.sync.dma_start(woT_sb, woT[:])

            bq_col = singles.tile([128, 1], F32)
            nc.sync.dma_start(bq_col, bq.rearrange("(p one) -> p one", one=1))
            bk_col = singles.tile([128, 1], F32)
            nc.sync.dma_start(bk_col, bk.rearrange("(p one) -> p one", one=1))
            bv_row = singles.tile([1, HD], F32)
            nc.sync.dma_start(bv_row, bv[None, :])

            ones_f32 = singles.tile([128, 128], F32)
            nc.vector.memset(ones_f32, 1.0)
            ones_bf = singles.tile([128, 4], BF16)
            nc.vector.tensor_copy(ones_bf, ones_f32[:, 0:4])

            # [128,128] causal triangle: tri[k, qq] = 1 if k <= qq.
            # Built in F32 (affine_select needs it), then cast.
            tri_f32 = singles.tile([128, 128], F32)
            nc.vector.memset(tri_f32, 1.0)
            nc.gpsimd.affine_select(
                out=tri_f32,
                in_=tri_f32,
                compare_op=mybir.AluOpType.is_ge,
                fill=0.0,
                base=0,
                pattern=[[1, 128]],
                channel_multiplier=-1,
            )
            tri_bf = singles.tile([128, 128], BF16)
            nc.vector.tensor_copy(tri_bf, tri_f32)

            # broadcast bv across partitions on gpsimd
            bias_v_sb = singles.tile([128, HD], F32)
            nc.gpsimd.partition_broadcast(bias_v_sb, bv_row)
            bias_v2 = bias_v_sb.rearrange("p (h x) -> p h x", h=2)

            # per-tile storage (separate tile objects -> precise deps)
            QT_t = [
                singles.tile([128, QT_TILE], BF16, name=f"qtt{i}", tag=f"qtt{i}")
                for i in range(N_QT)
            ]
            KT_t = [
                singles.tile([128, QT_TILE], BF16, name=f"ktt{i}", tag=f"ktt{i}")
                for i in range(N_QT)
            ]
            # V chunks in [k, d] layout; per tile: 4 chunks of
            # [V0 | ones | V1 | ones] (65-column stride per head slice)
            V_t = [
                singles.tile([128, 4, 130], BF16, name=f"vt{i}", tag=f"vt{i}")
                for i in range(N_QT)
            ]
            # unnormalized attention outputs + denominators (deferred norm)
            YTu_t = [
                [
                    singles.tile(
                        [64, QT_TILE], BF16, name=f"ytu{h}_{i}", tag=f"ytu{h}_{i}"
                    )
                    for i in range(N_QT)
                ]
                for h in range(2)
            ]
            den_t = [
                [
                    singles.tile([1, QT_TILE], F32, name=f"den{h}_{i}", tag=f"den{h}_{i}")
                    for i in range(N_QT)
                ]
                for h in range(2)
            ]
            # normalized YT, both heads stacked on partitions (h0: 0-63,
            # h1: 64-127) so the out-projection contracts K=128 in one shot
            YTn_t = [
                singles.tile([128, QT_TILE], BF16, name=f"ytn{i}", tag=f"ytn{i}")
                for i in range(N_QT)
            ]
            for i in range(N_QT):
                nc.vector.tensor_copy(V_t[i][:, :, 64:65], ones_bf[:, :, None])
                nc.vector.tensor_copy(V_t[i][:, :, 129:130], ones_bf[:, :, None])

            def emit_qproj(qt, xt):
                ps_q = ps_proj.tile([128, QT_TILE], F32, tag="psproj", name="ps_q")
                for kc in range(CK):
                    nc.tensor.matmul(
                        ps_q,
                        wqT_sb[:, kc, :],
                        xt[:, kc, :],
                        start=(kc == 0),
                        stop=(kc == CK - 1),
                    )
                nc.vector.tensor_scalar_add(QT_t[qt][:], ps_q, bq_col)

            def emit_kproj(qt, xt):
                ps_k = ps_proj.tile([128, QT_TILE], F32, tag="psproj", name="ps_k")
                for kc in range(CK):
                    nc.tensor.matmul(
                        ps_k,
                        wkT_sb[:, kc, :],
                        xt[:, kc, :],
                        start=(kc == 0),
                        stop=(kc == CK - 1),
                    )
                nc.vector.tensor_scalar_add(KT_t[qt][:], ps_k, bk_col)

            def emit_vproj(qt, xt, sv):
                ps_v = ps_proj.tile([128, HD], F32, tag="psproj", name="ps_v")
                for kc in range(CK):
                    nc.tensor.matmul(
                        ps_v,
                        xt[:, kc, bass.ts(sv, 128)],
                        wvT_sb[:, kc, :],
                        start=(kc == 0),
                        stop=(kc == CK - 1),
                    )
                vt = V_t[qt]
                v_vals = bass.AP(
                    tensor=vt.tensor,
                    offset=vt.offset,
                    ap=[vt.ap[0], vt.ap[1], [65, 2], [1, 64]],
                )
                nc.vector.tensor_add(
                    v_vals[:, sv],
                    ps_v.rearrange("p (h x) -> p h x", h=2),
                    bias_v2,
                )

            def emit_norm_a(qt, yt_ps):
                # evict yt_ps fast (den row + unnormalized YT), then
                # broadcast den across partitions on the idle gpsimd so
                # neither PE nor the DVE queue head ever waits on it.
                for h in range(2):
                    nc.vector.tensor_copy(den_t[h][qt][:], yt_ps[h][64:65, :])
                for h in range(2):
                    nc.vector.tensor_copy(YTu_t[h][qt], yt_ps[h][0:64, :])
                den_bc = []
                for h in range(2):
                    bc = norm.tile(
                        [64, QT_TILE], F32, tag=f"denbc{h}", name=f"denbc{h}"
                    )
                    nc.gpsimd.partition_broadcast(bc, den_t[h][qt][:])
                    den_bc.append(bc)
                return den_bc

            def emit_norm_b(qt, den_bc):
                # ~51-ULP reciprocal (5x faster than the iterative divide),
                # then scale the unnormalized attention rows.
                for h in range(2):
                    rec_sb = norm.tile(
                        [64, QT_TILE], F32, tag=f"rec{h}", name=f"rec{h}"
                    )
                    nc.vector.reciprocal_approx_fast(rec_sb, den_bc[h])
                    nc.vector.tensor_mul(
                        YTn_t[qt][64 * h : 64 * h + 64, :],
                        YTu_t[h][qt][:],
                        rec_sb,
                    )

            def emit_outproj_sv(qt, sv):
                tc8 = qt * (QT_TILE // 128) + sv
                ps_o = ps_proj.tile([128, C], F32, tag="psproj", name="ps_o")
                nc.tensor.matmul(
                    ps_o,
                    YTn_t[qt][:, bass.ts(sv, 128)],
                    woT_sb,
                    start=True,
                    stop=True,
                )
                o_sb = osb.tile([128, C], F32, tag="osb")
                nc.vector.tensor_copy(o_sb, ps_o)
                nc.sync.dma_start(out[bass.ts(tc8, 128), :], o_sb)

            xt_tiles = {0: xt_first}

            def emit_xt(i):
                if i not in xt_tiles and i < N_QT:
                    xt_i = xin.tile(
                        [128, CK, QT_TILE], BF16, tag="xt", name=f"xt{i}"
                    )
                    nc.sync.dma_start(xt_i, xT_ap[:, :, bass.ts(i, QT_TILE)])
                    xt_tiles[i] = xt_i

            def emit_s_exp(qt2, pair):
                """S^T quad (both heads, 2 chunks) + exp + causal mask for
                (query tile qt2, chunk pair). Diagonal chunks skip the
                fully-masked 128r query prefix (excluded from Y, never
                read) and mask only the [128,128] triangle block."""
                s_ps = [
                    ps_s.tile([128, 2, QT_TILE], F32, tag="s", name=f"s{h}")
                    for h in range(2)
                ]
                for sub in range(2):
                    c = pair * 2 + sub
                    r = c - 4 * qt2
                    off = KC * r if r > 0 else 0
                    for h in range(2):
                        hp = slice(h * 64, h * 64 + 64)
                        nc.tensor.matmul(
                            s_ps[h][:, sub, off:],
                            KT_t[c // 4][hp, bass.ts(c % 4, KC)],
                            QT_t[qt2][hp, off:],
                            start=True,
                            stop=True,
                        )
                e_sb = exps.tile([128, 2, 2, QT_TILE], BF16, tag="e", name="e")
                for h in range(2):
                    nc.scalar.activation(
                        e_sb[:, h],
                        s_ps[h],
                        mybir.ActivationFunctionType.Exp,
                        scale=SCALE,
                    )
                for sub in range(2):
                    c = pair * 2 + sub
                    r = c - 4 * qt2
                    if r >= 0:
                        for h in range(2):
                            nc.vector.tensor_mul(
                                e_sb[:, h, sub, bass.ts(r, KC)],
                                e_sb[:, h, sub, bass.ts(r, KC)],
                                tri_bf,
                            )
                return e_sb

            qproj_done = set()
            den_bcs = {}
            pending_e = {}
            for i in range(1, N_QT):
                emit_xt(i)
            for qt in range(N_QT):
                xt = xt_tiles[qt]
                if qt not in qproj_done:
                    emit_qproj(qt, xt)
                    qproj_done.add(qt)
                if qt == 0:
                    emit_kproj(qt, xt)
                    for sv in range(4):
                        emit_vproj(qt, xt, sv)

                yt_ps = [
                    ps_yt.tile([128, QT_TILE], F32, tag=f"yt{h}", name=f"yt{h}")
                    for h in range(2)
                ]
                n_pairs = 2 * (qt + 1)
                outproj_at = {}
                for sv in range(4):
                    outproj_at.setdefault(min(3 + sv, n_pairs - 1), []).append(sv)
                for pair in range(n_pairs):
                    e_sb = pending_e.pop((qt, pair), None)
                    if e_sb is None:
                        e_sb = emit_s_exp(qt, pair)
                    # pipelined projections / out-proj for other tiles
                    if pair == 0 and qt > 0:
                        emit_kproj(qt, xt)
                    if qt > 0 and pair < 4:
                        emit_vproj(qt, xt, pair)
                    if pair == min(2, n_pairs - 1) and qt + 1 < N_QT:
                        emit_qproj(qt + 1, xt_tiles[qt + 1])
                        qproj_done.add(qt + 1)
                    if pair == n_pairs - 1 and qt + 1 < N_QT:
                        # hoist the next tile's first S quad + exp so PE/ACT
                        # never idle across the tile boundary
                        pending_e[(qt + 1, 0)] = emit_s_exp(qt + 1, 0)
                    for h in range(2):
                        for sub in range(2):
                            c = pair * 2 + sub
                            r = c - 4 * qt
                            off = KC * r if r > 0 else 0
                            nc.tensor.matmul(
                                yt_ps[h][0:65, off:],
                                V_t[c // 4][:, c % 4, h * 65 : h * 65 + 65],
                                e_sb[:, h, sub, off:],
                                start=(pair == 0 and sub == 0),
                                stop=(pair == n_pairs - 1 and sub == 1),
                            )
                    if pair == 1 and qt > 0:
                        emit_norm_b(qt - 1, den_bcs[qt - 1])
                    if qt > 0:
                        for sv in outproj_at.get(pair, []):
                            emit_outproj_sv(qt - 1, sv)

                # ---- evict yt_ps fast + deferred normalization ----
                den_bcs[qt] = emit_norm_a(qt, yt_ps)
            emit_norm_b(N_QT - 1, den_bcs[N_QT - 1])
            for sv in range(4):
                emit_outproj_sv(N_QT - 1, sv)

    return nc


_PROGRAM = None


def _get_program():
    global _PROGRAM
    if _PROGRAM is None:
        _PROGRAM = build_program()
        if not _PROGRAM.is_finalized():
            _PROGRAM.finalize()
    return _PROGRAM


def make_in_maps(x, w_qkv, b_qkv, w_out, b_out):
    """Shard the full inputs into per-core input maps."""
    import ml_dtypes

    bf16 = ml_dtypes.bfloat16
    x = np.ascontiguousarray(x, dtype=np.float32)
    w_qkv = np.ascontiguousarray(w_qkv, dtype=np.float32)
    b_qkv = np.ascontiguousarray(b_qkv, dtype=np.float32)
    w_out = np.ascontiguousarray(w_out, dtype=np.float32)

    wq = w_qkv[0:C]  # [C, C] rows = q features
    wk = w_qkv[C : 2 * C]
    wv = w_qkv[2 * C : 3 * C]
    bq_full = b_qkv[0:C]
    bk_full = b_qkv[C : 2 * C]
    bv_full = b_qkv[2 * C : 3 * C]

    xT_b = [np.ascontiguousarray(x[b].T.astype(bf16)) for b in range(B)]

    in_maps = []
    for core in range(N_CORES):
        b = core // 4
        g = core % 4
        rows = slice(g * HD, (g + 1) * HD)  # this core's head dims
        woT = np.ascontiguousarray(w_out[:, rows].T.astype(bf16))  # [HD, C]
        in_maps.append(
            {
                "xT": xT_b[b],
                "wqT": np.ascontiguousarray(wq[rows].T.astype(bf16)),
                "wkT": np.ascontiguousarray(wk[rows].T.astype(bf16)),
                "wvT": np.ascontiguousarray(wv[rows].T.astype(bf16)),
                "woT": woT,
                "bq": np.ascontiguousarray(bq_full[rows]),
                "bk": np.ascontiguousarray(bk_full[rows]),
                "bv": np.ascontiguousarray(bv_full[rows]),
            }
        )
    return in_maps


def kernel(x, w_qkv, b_qkv, w_out, b_out, _trace=False, _trace_kwargs=None):
    in_maps = make_in_maps(x, w_qkv, b_qkv, w_out, b_out)
    nc = _get_program()
    res = run_bass_kernel_spmd(
        nc,
        in_maps,
        list(range(N_CORES)),
        trace=_trace,
        **(_trace_kwargs or {}),
    )
    outs = [res.results[c]["out"] for c in range(N_CORES)]
    bo = np.asarray(b_out, dtype=np.float32)
    # unshard: sum the 4 row-parallel partials per batch (+ bias), stack
    y = np.stack(
        [
            outs[0] + outs[1] + outs[2] + outs[3] + bo,
            outs[4] + outs[5] + outs[6] + outs[7] + bo,
        ]
    ).astype(np.float32)
    if _trace:
        return y, res
    return y
